# revision 31
# baseline (speedup 1.0000x reference)
"""Trainium2 Bass kernel for nn_Attention (additive-attention scoring).

Reference computation (per batch b):
    q[b]      = query[b] @ Wq.T + bq                       # [D]
    e[b,:,l]  = Wr @ ref[l,b,:] + br                       # [D, L]
    logits[b,l] = sum_o v[o] * tanh(q[b,o] + e[b,o,l])     # [L]
Returns (e, logits) with e: [B, D, L] f32, logits: [B, L] f32.

Strategy: data-parallel over batch - B=64 over 8 cores, weights replicated.
Each core's ref shard is laid out host-side as [B_loc, D, L] (the reference's
own ref_bdl view) during in_map construction, so the contraction dim D lands
on SBUF partitions with a plain strided DMA - no on-device transpose.
The conv matmul runs in float32r (full-rate fp32 on the PE at N>=256), f32
accumulate in PSUM. DVE adds br and stores e (f32); ACT computes
tanh(conv + q+bq+br) to bf16; a PE M=8 matmul against replicated-v reduces
over D for the logits.
"""

import os
import sys

import numpy as np

for _p in ("/root/.axon_site", "/root/.axon_site/_ro/trn_rl_repo",
           "/root/.axon_site/_ro/pypackages", "/opt/trn_rl_repo"):
    if os.path.isdir(_p) and _p not in sys.path:
        sys.path.append(_p)

import concourse.bass as bass  # noqa: E402,F401
import concourse.mybir as mybir  # noqa: E402
import concourse.tile as tile  # noqa: E402
from concourse import bacc  # noqa: E402
from concourse.bass_utils import run_bass_kernel_spmd  # noqa: E402
from concourse.masks import make_identity  # noqa: E402

F32 = mybir.dt.float32
F32R = mybir.dt.float32r
BF16 = mybir.dt.bfloat16

NCORES = 8
B = 64
BLOC = B // NCORES  # 8 batches per core
D = 512
L = 2048
DT = D // 128  # 4 partition tiles of the dim axis
LC = 512      # l-chunk processed per inner iteration
NLC = L // LC  # 4


def _preamble(nc, tc, singles, query, Wq, bq, Wr, br, v):
    """Load + transpose weights, compute q = Linear(query). Returns the
    SBUF residents used by the main loop."""
    identity = singles.tile([128, 128], F32)
    make_identity(nc, identity)

    # Per-partition columns of the small vectors: col[p, O] = vec[O*128+p]
    # stripe-permuted: col[p, r] = vec[4p + r] (matches the weight layout;
    # fully contiguous DMA)
    br_col = singles.tile([128, DT], F32)
    nc.gpsimd.dma_start(out=br_col[:], in_=br[:].rearrange("(p r) -> p r", p=128))
    bq_col = singles.tile([128, DT], F32)
    nc.gpsimd.dma_start(out=bq_col[:], in_=bq[:].rearrange("(p r) -> p r", p=128))
    v_col = singles.tile([128, DT], F32)
    nc.gpsimd.dma_start(out=v_col[:], in_=v[:].rearrange("(p r) -> p r", p=128))
    bqbr_col = singles.tile([128, DT], F32)
    nc.vector.tensor_add(bqbr_col[:], bq_col[:], br_col[:])

    # v replicated across BLOC columns -> stationary operand of the
    # v-reduce matmul (M=BLOC so output rows land on partitions 0..7).
    zero_bloc = singles.tile([128, BLOC], F32)
    nc.vector.memset(zero_bloc[:], 0.0)
    v_rep = singles.tile([128, DT, BLOC], BF16)
    for O in range(DT):
        nc.vector.tensor_scalar_add(v_rep[:, O, :], zero_bloc[:], v_col[:, O : O + 1])

    # Transposed weights, stripe-permuted output rows:
    # WT[i_part, I, r, p] = W[4p + r, I*128 + i_part] - a matmul with this
    # lhsT produces PSUM partition p holding output row o = 4p + r; the
    # permutation is undone by an affine store AP (stride 4 rows).
    WrT_sb = singles.tile([128, DT, DT, 128], BF16)
    qbr_sb = singles.tile([128, DT, BLOC], F32)

    with (
        tc.tile_pool(name="pre", bufs=2) as pre,
        tc.tile_pool(name="pre_ps", bufs=2, space="PSUM") as pre_ps,
    ):
        # WqT only lives for the preamble - keep it in the scoped pool
        WqT_sb = pre.tile([128, DT, DT, 128], F32, tag="wqt", bufs=1, name="WqT_sb")
        for W_dram, WT_sb in ((Wr, WrT_sb), (Wq, WqT_sb)):
            W_nat = pre.tile([128, DT, D], F32, tag="wnat", bufs=1, name="W_nat")
            # o-rows 4p..4p+3 are DRAM-contiguous: one 8 KB run/partition
            nc.scalar.dma_start(
                out=W_nat[:], in_=W_dram[:, :].rearrange("(p r) i -> p r i", p=128)
            )
            for r in range(DT):
                W_ps = pre_ps.tile([128, D], F32, tag="wps", name="W_ps")
                for I in range(DT):
                    nc.tensor.transpose(
                        W_ps[:, I * 128 : (I + 1) * 128],
                        W_nat[:, r, I * 128 : (I + 1) * 128],
                        identity[:],
                    )
                nc.vector.tensor_copy(
                    WT_sb[:, :, r, :],
                    W_ps[:].rearrange("p (I c) -> p I c", I=DT),
                )

        # queryT[i, b] on partitions, then q = Wq.T-tiles @ queryT
        q_nat = pre.tile([BLOC, D], F32, tag="qnat")
        nc.scalar.dma_start(out=q_nat[:], in_=query[:, :])
        qT_ps = pre_ps.tile([128, DT, BLOC], F32, tag="qtps")
        for I in range(DT):
            nc.tensor.transpose(
                qT_ps[:, I, :],
                q_nat[:, I * 128 : (I + 1) * 128],
                identity[0:BLOC, 0:BLOC],
            )
        qT_sb = pre.tile([128, DT, BLOC], F32, tag="qtsb")
        nc.vector.tensor_copy(qT_sb[:], qT_ps[:])
        for O in range(DT):
            q_ps = pre_ps.tile([128, BLOC], F32, tag="qps", name="q_ps")
            for I in range(DT):
                nc.tensor.matmul(
                    q_ps[:],
                    WqT_sb[:, I, O, :],
                    qT_sb[:, I, :],
                    start=(I == 0),
                    stop=(I == DT - 1),
                )
            # qbr = q + bq + br (the tanh bias; e's br is added separately)
            nc.vector.tensor_scalar_add(qbr_sb[:, O, :], q_ps[:], bqbr_col[:, O : O + 1])

    return identity, br_col, v_rep, WrT_sb, qbr_sb


def build_nc():
    nc = bacc.Bacc(None)

    query = nc.declare_dram_parameter("query", [BLOC, D], F32, isOutput=False)
    # ref arrives pre-permuted host-side to [B_loc, D, L] (the reference's own
    # ref_bdl view) so D lands on partitions with a plain strided load
    ref = nc.declare_dram_parameter("ref", [BLOC, D, L], F32, isOutput=False)
    Wq = nc.declare_dram_parameter("Wq", [D, D], F32, isOutput=False)
    bq = nc.declare_dram_parameter("bq", [D], F32, isOutput=False)
    Wr = nc.declare_dram_parameter("Wr", [D, D], F32, isOutput=False)
    br = nc.declare_dram_parameter("br", [D], F32, isOutput=False)
    v = nc.declare_dram_parameter("v", [D], F32, isOutput=False)
    e_out = nc.declare_dram_parameter("e", [BLOC, D, L], F32, isOutput=True)
    logits_out = nc.declare_dram_parameter("logits", [BLOC, L], F32, isOutput=True)

    with tile.TileContext(nc) as tc:
        with (
            tc.tile_pool(name="singles", bufs=1) as singles,
            tc.tile_pool(name="rpool", bufs=3) as rpool,
            tc.tile_pool(name="epool", bufs=6) as epool,
            tc.tile_pool(name="ttpool", bufs=17) as ttpool,
            tc.tile_pool(name="lpool", bufs=2) as lpool,
        ):
            def issue_load(b):
                # four contiguous 1 MB stripe loads, f32 -> bf16 cast in
                # flight (SWDGE): i = I*128 + p
                R_T = rpool.tile([128, DT, L], BF16, tag="rT", name=f"RT{b}")
                for I in range(DT):
                    nc.gpsimd.dma_start(
                        out=R_T[:, I, :], in_=ref[b, I * 128 : (I + 1) * 128, :]
                    )
                return R_T

            # start streaming ref before the preamble so the first conv
            # matmuls are not gated on the preamble pool-exit barrier
            R_tiles = {0: issue_load(0), 1: issue_load(1)}

            identity, br_col, v_rep, WrT_sb, qbr_sb = _preamble(
                nc, tc, singles, query, Wq, bq, Wr, br, v
            )

            with (
                tc.tile_pool(name="cps", bufs=6, space="PSUM") as cps_pool,
                tc.tile_pool(name="ups", bufs=2, space="PSUM") as ups_pool,
            ):
                for b in range(BLOC):
                    if b + 2 < BLOC:
                        R_tiles[b + 2] = issue_load(b + 2)
                    R_T = R_tiles[b]
                    e_sbs = [
                        epool.tile([128, L], F32, tag="esb", name=f"esb{b}_{O}")
                        for O in range(DT)
                    ]
                    logit_b = lpool.tile([1, L], F32, tag="lsb", name=f"lsb{b}")

                    # conv: keep each weight block stationary across all 4
                    # l-chunks (one LDWEIGHTS per 4 matmuls - FWL is off for
                    # f32r, so weight loads are the PE overhead to amortize)
                    t_grid = {}
                    for O in range(DT):
                        C_lcs = [
                            cps_pool.tile([128, LC], F32, tag="cps", name=f"C{b}_{O}_{lc}")
                            for lc in range(NLC)
                        ]
                        for I in range(DT):
                            for lc in range(NLC):
                                nc.tensor.matmul(
                                    C_lcs[lc][:],
                                    WrT_sb[:, I, O, :],
                                    R_T[:, I, lc * LC : (lc + 1) * LC],
                                    start=(I == 0),
                                    stop=(I == DT - 1),
                                )
                        for lc in range(NLC):
                            nc.vector.tensor_scalar_add(
                                e_sbs[O][:, lc * LC : (lc + 1) * LC],
                                C_lcs[lc][:],
                                br_col[:, O : O + 1],
                            )
                            t_sb = ttpool.tile([128, LC], BF16, tag="ttsb", name="t_sb")
                            nc.scalar.activation(
                                t_sb[:],
                                C_lcs[lc][:],
                                mybir.ActivationFunctionType.Tanh,
                                bias=qbr_sb[:, O, b : b + 1],
                            )
                            t_grid[(O, lc)] = t_sb
                        # store this stripe immediately (partition p ->
                        # row 4p+O, affine stride); alternate rings
                        eng = nc.sync if (b + O) % 2 == 0 else nc.scalar
                        eng.dma_start(
                            out=e_out[b, :, :].rearrange("(p r) l -> p r l", p=128)[
                                :, O, :
                            ],
                            in_=e_sbs[O][:],
                        )

                    for lc in range(NLC):
                        U_ps = ups_pool.tile([BLOC, LC], F32, tag="ups", name="U_ps")
                        for O in range(DT):
                            nc.tensor.matmul(
                                U_ps[:],
                                v_rep[:, O, :],
                                t_grid[(O, lc)][:],
                                start=(O == 0),
                                stop=(O == DT - 1),
                            )
                        nc.vector.tensor_copy(
                            logit_b[0:1, lc * LC : (lc + 1) * LC], U_ps[0:1, :]
                        )

                    nc.scalar.dma_start(out=logits_out[b, :], in_=logit_b[0:1, :])
                    del R_tiles[b]


    nc.compile()
    return nc


_CACHE: dict = {}


def _get_nc():
    if "nc" not in _CACHE:
        _CACHE["nc"] = build_nc()
    return _CACHE["nc"]


def kernel(**inputs) -> tuple[np.ndarray, np.ndarray]:
    query = np.ascontiguousarray(np.asarray(inputs["query"], dtype=np.float32))
    ref = np.asarray(inputs["ref"], dtype=np.float32)
    Wq = np.ascontiguousarray(np.asarray(inputs["Wq"], dtype=np.float32))
    bq = np.ascontiguousarray(np.asarray(inputs["bq"], dtype=np.float32))
    Wr = np.ascontiguousarray(np.asarray(inputs["Wr"], dtype=np.float32))
    br = np.ascontiguousarray(np.asarray(inputs["br"], dtype=np.float32))
    v = np.ascontiguousarray(np.asarray(inputs["v"], dtype=np.float32))

    in_maps = []
    for c in range(NCORES):
        sl = slice(c * BLOC, (c + 1) * BLOC)
        in_maps.append(
            {
                "query": np.ascontiguousarray(query[sl]),
                # shard + lay out as [B_loc, D, L] (the reference's ref_bdl
                # view) so the device reads D directly onto partitions
                "ref": np.ascontiguousarray(np.transpose(ref[:, sl, :], (1, 2, 0))),
                "Wq": Wq,
                "bq": bq,
                "Wr": Wr,
                "br": br,
                "v": v,
            }
        )

    res = run_bass_kernel_spmd(_get_nc(), in_maps, core_ids=list(range(NCORES)))
    _CACHE["last_result"] = res

    e = np.concatenate([res.results[c]["e"] for c in range(NCORES)], axis=0)
    logits = np.concatenate([res.results[c]["logits"] for c in range(NCORES)], axis=0)
    return (e, logits)


# revision 32
# speedup vs baseline: 1.0251x; 1.0251x over previous
"""Trainium2 Bass kernel for nn_Attention (additive-attention scoring).

Reference computation (per batch b):
    q[b]      = query[b] @ Wq.T + bq                       # [D]
    e[b,:,l]  = Wr @ ref[l,b,:] + br                       # [D, L]
    logits[b,l] = sum_o v[o] * tanh(q[b,o] + e[b,o,l])     # [L]
Returns (e, logits) with e: [B, D, L] f32, logits: [B, L] f32.

Strategy: data-parallel over batch - B=64 over 8 cores, weights replicated.
Each core's ref shard is laid out host-side as [B_loc, D, L] (the reference's
own ref_bdl view) during in_map construction, so the contraction dim D lands
on SBUF partitions with a plain strided DMA - no on-device transpose.
The conv matmul runs in float32r (full-rate fp32 on the PE at N>=256), f32
accumulate in PSUM. DVE adds br and stores e (f32); ACT computes
tanh(conv + q+bq+br) to bf16; a PE M=8 matmul against replicated-v reduces
over D for the logits.
"""

import os
import sys

import numpy as np

for _p in ("/root/.axon_site", "/root/.axon_site/_ro/trn_rl_repo",
           "/root/.axon_site/_ro/pypackages", "/opt/trn_rl_repo"):
    if os.path.isdir(_p) and _p not in sys.path:
        sys.path.append(_p)

import concourse.bass as bass  # noqa: E402,F401
import concourse.mybir as mybir  # noqa: E402
import concourse.tile as tile  # noqa: E402
from concourse import bacc  # noqa: E402
from concourse.bass_utils import run_bass_kernel_spmd  # noqa: E402
from concourse.masks import make_identity  # noqa: E402

F32 = mybir.dt.float32
F32R = mybir.dt.float32r
BF16 = mybir.dt.bfloat16

NCORES = 8
B = 64
BLOC = B // NCORES  # 8 batches per core
D = 512
L = 2048
DT = D // 128  # 4 partition tiles of the dim axis
LC = 512      # l-chunk processed per inner iteration
NLC = L // LC  # 4


def _preamble(nc, tc, singles, query, Wq, bq, Wr, br, v):
    """Load + transpose weights, compute q = Linear(query). Returns the
    SBUF residents used by the main loop."""
    identity = singles.tile([128, 128], F32)
    make_identity(nc, identity)

    # Per-partition columns of the small vectors: col[p, O] = vec[O*128+p]
    # stripe-permuted: col[p, r] = vec[4p + r] (matches the weight layout;
    # fully contiguous DMA)
    br_col = singles.tile([128, DT], F32)
    nc.gpsimd.dma_start(out=br_col[:], in_=br[:].rearrange("(p r) -> p r", p=128))
    bq_col = singles.tile([128, DT], F32)
    nc.gpsimd.dma_start(out=bq_col[:], in_=bq[:].rearrange("(p r) -> p r", p=128))
    v_col = singles.tile([128, DT], F32)
    nc.gpsimd.dma_start(out=v_col[:], in_=v[:].rearrange("(p r) -> p r", p=128))
    bqbr_col = singles.tile([128, DT], F32)
    nc.vector.tensor_add(bqbr_col[:], bq_col[:], br_col[:])

    # v replicated across BLOC columns -> stationary operand of the
    # v-reduce matmul (M=BLOC so output rows land on partitions 0..7).
    zero_bloc = singles.tile([128, BLOC], F32)
    nc.vector.memset(zero_bloc[:], 0.0)
    v_rep = singles.tile([128, DT, BLOC], BF16)
    for O in range(DT):
        nc.vector.tensor_scalar_add(v_rep[:, O, :], zero_bloc[:], v_col[:, O : O + 1])

    # Transposed weights, stripe-permuted output rows:
    # WT[i_part, I, r, p] = W[4p + r, I*128 + i_part] - a matmul with this
    # lhsT produces PSUM partition p holding output row o = 4p + r; the
    # permutation is undone by an affine store AP (stride 4 rows).
    WrT_sb = singles.tile([128, DT, DT, 128], F32R)
    qbr_sb = singles.tile([128, DT, BLOC], F32)

    with (
        tc.tile_pool(name="pre", bufs=2) as pre,
        tc.tile_pool(name="pre_ps", bufs=2, space="PSUM") as pre_ps,
    ):
        # WqT only lives for the preamble - keep it in the scoped pool
        WqT_sb = pre.tile([128, DT, DT, 128], F32, tag="wqt", bufs=1, name="WqT_sb")
        # query first: the q-chain occupies the head of the PE queue, so
        # everything it needs must land before the ref stripes do
        q_nat = pre.tile([BLOC, D], F32, tag="qnat")
        nc.scalar.dma_start(out=q_nat[:], in_=query[:, :])

        for W_dram, WT_sb in ((Wq, WqT_sb), (Wr, WrT_sb)):
            W_nat = pre.tile([128, DT, D], F32, tag="wnat", bufs=1, name="W_nat")
            # o-rows 4p..4p+3 are DRAM-contiguous: one 8 KB run/partition
            nc.scalar.dma_start(
                out=W_nat[:], in_=W_dram[:, :].rearrange("(p r) i -> p r i", p=128)
            )
            for r in range(DT):
                W_ps = pre_ps.tile([128, D], F32, tag="wps", name="W_ps")
                for I in range(DT):
                    nc.tensor.transpose(
                        W_ps[:, I * 128 : (I + 1) * 128],
                        W_nat[:, r, I * 128 : (I + 1) * 128],
                        identity[:],
                    )
                nc.vector.tensor_copy(
                    WT_sb[:, :, r, :],
                    W_ps[:].rearrange("p (I c) -> p I c", I=DT),
                )

        # queryT[i, b] on partitions, then q = Wq.T-tiles @ queryT
        qT_ps = pre_ps.tile([128, DT, BLOC], F32, tag="qtps")
        for I in range(DT):
            nc.tensor.transpose(
                qT_ps[:, I, :],
                q_nat[:, I * 128 : (I + 1) * 128],
                identity[0:BLOC, 0:BLOC],
            )
        qT_sb = pre.tile([128, DT, BLOC], F32, tag="qtsb")
        nc.vector.tensor_copy(qT_sb[:], qT_ps[:])
        for O in range(DT):
            q_ps = pre_ps.tile([128, BLOC], F32, tag="qps", name="q_ps")
            for I in range(DT):
                nc.tensor.matmul(
                    q_ps[:],
                    WqT_sb[:, I, O, :],
                    qT_sb[:, I, :],
                    start=(I == 0),
                    stop=(I == DT - 1),
                )
            # qbr = q + bq + br (the tanh bias; e's br is added separately)
            nc.vector.tensor_scalar_add(qbr_sb[:, O, :], q_ps[:], bqbr_col[:, O : O + 1])

    return identity, br_col, v_rep, WrT_sb, qbr_sb


def build_nc():
    nc = bacc.Bacc(None)

    query = nc.declare_dram_parameter("query", [BLOC, D], F32, isOutput=False)
    # ref arrives pre-permuted host-side to [B_loc, D, L] (the reference's own
    # ref_bdl view) so D lands on partitions with a plain strided load
    ref = nc.declare_dram_parameter("ref", [BLOC, D, L], F32R, isOutput=False)
    Wq = nc.declare_dram_parameter("Wq", [D, D], F32, isOutput=False)
    bq = nc.declare_dram_parameter("bq", [D], F32, isOutput=False)
    Wr = nc.declare_dram_parameter("Wr", [D, D], F32, isOutput=False)
    br = nc.declare_dram_parameter("br", [D], F32, isOutput=False)
    v = nc.declare_dram_parameter("v", [D], F32, isOutput=False)
    e_out = nc.declare_dram_parameter("e", [BLOC, D, L], F32, isOutput=True)
    logits_out = nc.declare_dram_parameter("logits", [BLOC, L], F32, isOutput=True)

    with tile.TileContext(nc) as tc:
        with (
            tc.tile_pool(name="singles", bufs=1) as singles,
            tc.tile_pool(name="rpool", bufs=3) as rpool,
            tc.tile_pool(name="epool", bufs=6) as epool,
            tc.tile_pool(name="ttpool", bufs=17) as ttpool,
            tc.tile_pool(name="lpool", bufs=2) as lpool,
        ):
            def issue_load(b):
                # four fully-contiguous 1 MB stripe loads: i = I*128 + p
                R_T = rpool.tile([128, DT, L], F32R, tag="rT", name=f"RT{b}")
                for I in range(DT):
                    nc.sync.dma_start(
                        out=R_T[:, I, :], in_=ref[b, I * 128 : (I + 1) * 128, :]
                    )
                return R_T

            # start streaming ref before the preamble so the first conv
            # matmuls are not gated on the preamble pool-exit barrier
            R_tiles = {0: issue_load(0), 1: issue_load(1)}

            identity, br_col, v_rep, WrT_sb, qbr_sb = _preamble(
                nc, tc, singles, query, Wq, bq, Wr, br, v
            )

            with (
                tc.tile_pool(name="cps", bufs=6, space="PSUM") as cps_pool,
                tc.tile_pool(name="ups", bufs=2, space="PSUM") as ups_pool,
            ):
                for b in range(BLOC):
                    if b + 2 < BLOC:
                        R_tiles[b + 2] = issue_load(b + 2)
                    R_T = R_tiles[b]
                    e_sbs = [
                        epool.tile([128, L], F32, tag="esb", name=f"esb{b}_{O}")
                        for O in range(DT)
                    ]
                    # All BLOC rows of U_ps are identical (v replicated), so
                    # logits live on partition 0 - PSUM reads must start at an
                    # aligned partition, so row b is never read directly.
                    logit_b = lpool.tile([1, L], F32, tag="lsb", name=f"lsb{b}")

                    # conv: keep each weight block stationary across all 4
                    # l-chunks (one LDWEIGHTS per 4 matmuls - FWL is off for
                    # f32r, so weight loads are the PE overhead to amortize)
                    t_grid = {}
                    for O in range(DT):
                        C_lcs = [
                            cps_pool.tile([128, LC], F32, tag="cps", name=f"C{b}_{O}_{lc}")
                            for lc in range(NLC)
                        ]
                        for I in range(DT):
                            for lc in range(NLC):
                                nc.tensor.matmul(
                                    C_lcs[lc][:],
                                    WrT_sb[:, I, O, :],
                                    R_T[:, I, lc * LC : (lc + 1) * LC],
                                    start=(I == 0),
                                    stop=(I == DT - 1),
                                )
                        for lc in range(NLC):
                            nc.vector.tensor_scalar_add(
                                e_sbs[O][:, lc * LC : (lc + 1) * LC],
                                C_lcs[lc][:],
                                br_col[:, O : O + 1],
                            )
                            t_sb = ttpool.tile([128, LC], BF16, tag="ttsb", name="t_sb")
                            nc.scalar.activation(
                                t_sb[:],
                                C_lcs[lc][:],
                                mybir.ActivationFunctionType.Tanh,
                                bias=qbr_sb[:, O, b : b + 1],
                            )
                            t_grid[(O, lc)] = t_sb
                        # store this stripe immediately (partition p ->
                        # row 4p+O, affine stride); alternate rings
                        eng = nc.gpsimd if (b + O) % 2 == 0 else nc.scalar
                        eng.dma_start(
                            out=e_out[b, :, :].rearrange("(p r) l -> p r l", p=128)[
                                :, O, :
                            ],
                            in_=e_sbs[O][:],
                        )

                    for lc in range(NLC):
                        U_ps = ups_pool.tile([BLOC, LC], F32, tag="ups", name="U_ps")
                        for O in range(DT):
                            nc.tensor.matmul(
                                U_ps[:],
                                v_rep[:, O, :],
                                t_grid[(O, lc)][:],
                                start=(O == 0),
                                stop=(O == DT - 1),
                            )
                        nc.vector.tensor_copy(
                            logit_b[0:1, lc * LC : (lc + 1) * LC], U_ps[0:1, :]
                        )
                        if b == BLOC - 1:
                            nc.scalar.dma_start(
                                out=logits_out[b, lc * LC : (lc + 1) * LC],
                                in_=logit_b[0:1, lc * LC : (lc + 1) * LC],
                            )

                    if b < BLOC - 1:
                        nc.scalar.dma_start(out=logits_out[b, :], in_=logit_b[0:1, :])
                    del R_tiles[b]

    nc.compile()
    return nc


_CACHE: dict = {}


def _get_nc():
    if "nc" not in _CACHE:
        _CACHE["nc"] = build_nc()
    return _CACHE["nc"]


def kernel(**inputs) -> tuple[np.ndarray, np.ndarray]:
    query = np.ascontiguousarray(np.asarray(inputs["query"], dtype=np.float32))
    ref = np.asarray(inputs["ref"], dtype=np.float32)
    Wq = np.ascontiguousarray(np.asarray(inputs["Wq"], dtype=np.float32))
    bq = np.ascontiguousarray(np.asarray(inputs["bq"], dtype=np.float32))
    Wr = np.ascontiguousarray(np.asarray(inputs["Wr"], dtype=np.float32))
    br = np.ascontiguousarray(np.asarray(inputs["br"], dtype=np.float32))
    v = np.ascontiguousarray(np.asarray(inputs["v"], dtype=np.float32))

    in_maps = []
    for c in range(NCORES):
        sl = slice(c * BLOC, (c + 1) * BLOC)
        in_maps.append(
            {
                "query": np.ascontiguousarray(query[sl]),
                # shard + lay out as [B_loc, D, L] (the reference's ref_bdl
                # view) so the device reads D directly onto partitions
                "ref": np.ascontiguousarray(np.transpose(ref[:, sl, :], (1, 2, 0))),
                "Wq": Wq,
                "bq": bq,
                "Wr": Wr,
                "br": br,
                "v": v,
            }
        )

    res = run_bass_kernel_spmd(_get_nc(), in_maps, core_ids=list(range(NCORES)))
    _CACHE["last_result"] = res

    e = np.concatenate([res.results[c]["e"] for c in range(NCORES)], axis=0)
    logits = np.concatenate([res.results[c]["logits"] for c in range(NCORES)], axis=0)
    return (e, logits)


# revision 33
# speedup vs baseline: 1.0700x; 1.0438x over previous
"""Trainium2 Bass kernel for nn_Attention (additive-attention scoring).

Reference computation (per batch b):
    q[b]      = query[b] @ Wq.T + bq                       # [D]
    e[b,:,l]  = Wr @ ref[l,b,:] + br                       # [D, L]
    logits[b,l] = sum_o v[o] * tanh(q[b,o] + e[b,o,l])     # [L]
Returns (e, logits) with e: [B, D, L] f32, logits: [B, L] f32.

Strategy: data-parallel over batch - B=64 over 8 cores, weights replicated.
Each core's ref shard is laid out host-side as [B_loc, D, L] (the reference's
own ref_bdl view) during in_map construction, so the contraction dim D lands
on SBUF partitions with a plain strided DMA - no on-device transpose.
The conv matmul runs in float32r (full-rate fp32 on the PE at N>=256), f32
accumulate in PSUM. DVE adds br and stores e (f32); ACT computes
tanh(conv + q+bq+br) to bf16; a PE M=8 matmul against replicated-v reduces
over D for the logits.
"""

import os
import sys

import numpy as np

for _p in ("/root/.axon_site", "/root/.axon_site/_ro/trn_rl_repo",
           "/root/.axon_site/_ro/pypackages", "/opt/trn_rl_repo"):
    if os.path.isdir(_p) and _p not in sys.path:
        sys.path.append(_p)

import concourse.bass as bass  # noqa: E402,F401
import concourse.mybir as mybir  # noqa: E402
import concourse.tile as tile  # noqa: E402
from concourse import bacc  # noqa: E402
from concourse.bass_utils import run_bass_kernel_spmd  # noqa: E402
from concourse.masks import make_identity  # noqa: E402

F32 = mybir.dt.float32
F32R = mybir.dt.float32r
BF16 = mybir.dt.bfloat16

NCORES = 8
B = 64
BLOC = B // NCORES  # 8 batches per core
D = 512
L = 2048
DT = D // 128  # 4 partition tiles of the dim axis
LC = 512      # l-chunk processed per inner iteration
NLC = L // LC  # 4


def _preamble(nc, tc, singles, query, Wq, bq, Wr, br, v):
    """Load + transpose weights, compute q = Linear(query). Returns the
    SBUF residents used by the main loop."""
    identity = singles.tile([128, 128], F32)
    make_identity(nc, identity)

    # Per-partition columns of the small vectors: col[p, O] = vec[O*128+p]
    # stripe-permuted: col[p, r] = vec[4p + r] (matches the weight layout;
    # fully contiguous DMA)
    br_col = singles.tile([128, DT], F32)
    nc.gpsimd.dma_start(out=br_col[:], in_=br[:].rearrange("(p r) -> p r", p=128))
    bq_col = singles.tile([128, DT], F32)
    nc.gpsimd.dma_start(out=bq_col[:], in_=bq[:].rearrange("(p r) -> p r", p=128))
    v_col = singles.tile([128, DT], F32)
    nc.gpsimd.dma_start(out=v_col[:], in_=v[:].rearrange("(p r) -> p r", p=128))
    bqbr_col = singles.tile([128, DT], F32)
    nc.vector.tensor_add(bqbr_col[:], bq_col[:], br_col[:])

    # v replicated across BLOC columns -> stationary operand of the
    # v-reduce matmul (M=BLOC so output rows land on partitions 0..7).
    zero_bloc = singles.tile([128, BLOC], F32)
    nc.vector.memset(zero_bloc[:], 0.0)
    v_rep = singles.tile([128, DT, BLOC], BF16)
    for O in range(DT):
        nc.vector.tensor_scalar_add(v_rep[:, O, :], zero_bloc[:], v_col[:, O : O + 1])

    # Transposed weights, stripe-permuted output rows:
    # WT[i_part, I, r, p] = W[4p + r, I*128 + i_part] - a matmul with this
    # lhsT produces PSUM partition p holding output row o = 4p + r; the
    # permutation is undone by an affine store AP (stride 4 rows).
    WrT_sb = singles.tile([128, DT, DT, 128], F32R)
    qbr_sb = singles.tile([128, DT, BLOC], F32)

    with (
        tc.tile_pool(name="pre", bufs=2) as pre,
        tc.tile_pool(name="pre_ps", bufs=2, space="PSUM") as pre_ps,
    ):
        # WqT only lives for the preamble - keep it in the scoped pool
        WqT_sb = pre.tile([128, DT, DT, 128], F32, tag="wqt", bufs=1, name="WqT_sb")
        for W_dram, WT_sb in ((Wr, WrT_sb), (Wq, WqT_sb)):
            W_nat = pre.tile([128, DT, D], F32, tag="wnat", bufs=1, name="W_nat")
            # o-rows 4p..4p+3 are DRAM-contiguous: one 8 KB run/partition
            nc.scalar.dma_start(
                out=W_nat[:], in_=W_dram[:, :].rearrange("(p r) i -> p r i", p=128)
            )
            for r in range(DT):
                W_ps = pre_ps.tile([128, D], F32, tag="wps", name="W_ps")
                for I in range(DT):
                    nc.tensor.transpose(
                        W_ps[:, I * 128 : (I + 1) * 128],
                        W_nat[:, r, I * 128 : (I + 1) * 128],
                        identity[:],
                    )
                nc.vector.tensor_copy(
                    WT_sb[:, :, r, :],
                    W_ps[:].rearrange("p (I c) -> p I c", I=DT),
                )

        # queryT[i, b] on partitions, then q = Wq.T-tiles @ queryT
        q_nat = pre.tile([BLOC, D], F32, tag="qnat")
        nc.scalar.dma_start(out=q_nat[:], in_=query[:, :])
        qT_ps = pre_ps.tile([128, DT, BLOC], F32, tag="qtps")
        for I in range(DT):
            nc.tensor.transpose(
                qT_ps[:, I, :],
                q_nat[:, I * 128 : (I + 1) * 128],
                identity[0:BLOC, 0:BLOC],
            )
        qT_sb = pre.tile([128, DT, BLOC], F32, tag="qtsb")
        nc.vector.tensor_copy(qT_sb[:], qT_ps[:])
        for O in range(DT):
            q_ps = pre_ps.tile([128, BLOC], F32, tag="qps", name="q_ps")
            for I in range(DT):
                nc.tensor.matmul(
                    q_ps[:],
                    WqT_sb[:, I, O, :],
                    qT_sb[:, I, :],
                    start=(I == 0),
                    stop=(I == DT - 1),
                )
            # qbr = q + bq + br (the tanh bias; e's br is added separately)
            nc.vector.tensor_scalar_add(qbr_sb[:, O, :], q_ps[:], bqbr_col[:, O : O + 1])

    return identity, br_col, v_rep, WrT_sb, qbr_sb


def build_nc():
    nc = bacc.Bacc(None)

    query = nc.declare_dram_parameter("query", [BLOC, D], F32, isOutput=False)
    # ref arrives pre-permuted host-side to [B_loc, D, L] (the reference's own
    # ref_bdl view) so D lands on partitions with a plain strided load
    ref = nc.declare_dram_parameter("ref", [BLOC, D, L], F32R, isOutput=False)
    Wq = nc.declare_dram_parameter("Wq", [D, D], F32, isOutput=False)
    bq = nc.declare_dram_parameter("bq", [D], F32, isOutput=False)
    Wr = nc.declare_dram_parameter("Wr", [D, D], F32, isOutput=False)
    br = nc.declare_dram_parameter("br", [D], F32, isOutput=False)
    v = nc.declare_dram_parameter("v", [D], F32, isOutput=False)
    e_out = nc.declare_dram_parameter("e", [BLOC, D, L], F32, isOutput=True)
    logits_out = nc.declare_dram_parameter("logits", [BLOC, L], F32, isOutput=True)

    with tile.TileContext(nc) as tc:
        with (
            tc.tile_pool(name="singles", bufs=1) as singles,
            tc.tile_pool(name="rpool", bufs=3) as rpool,
            tc.tile_pool(name="epool", bufs=6) as epool,
            tc.tile_pool(name="ttpool", bufs=17) as ttpool,
            tc.tile_pool(name="lpool", bufs=2) as lpool,
        ):
            def issue_load(b):
                # four fully-contiguous 1 MB stripe loads: i = I*128 + p
                R_T = rpool.tile([128, DT, L], F32R, tag="rT", name=f"RT{b}")
                for I in range(DT):
                    nc.sync.dma_start(
                        out=R_T[:, I, :], in_=ref[b, I * 128 : (I + 1) * 128, :]
                    )
                return R_T

            # start streaming ref before the preamble so the first conv
            # matmuls are not gated on the preamble pool-exit barrier
            R_tiles = {0: issue_load(0), 1: issue_load(1)}

            identity, br_col, v_rep, WrT_sb, qbr_sb = _preamble(
                nc, tc, singles, query, Wq, bq, Wr, br, v
            )

            with (
                tc.tile_pool(name="cps", bufs=6, space="PSUM") as cps_pool,
                tc.tile_pool(name="ups", bufs=2, space="PSUM") as ups_pool,
            ):
                for b in range(BLOC):
                    if b + 2 < BLOC:
                        R_tiles[b + 2] = issue_load(b + 2)
                    R_T = R_tiles[b]
                    e_sbs = [
                        epool.tile([128, L], F32, tag="esb", name=f"esb{b}_{O}")
                        for O in range(DT)
                    ]
                    # All BLOC rows of U_ps are identical (v replicated), so
                    # logits live on partition 0 - PSUM reads must start at an
                    # aligned partition, so row b is never read directly.
                    logit_b = lpool.tile([1, L], F32, tag="lsb", name=f"lsb{b}")

                    # conv: keep each weight block stationary across all 4
                    # l-chunks (one LDWEIGHTS per 4 matmuls - FWL is off for
                    # f32r, so weight loads are the PE overhead to amortize)
                    t_grid = {}
                    for O in range(DT):
                        C_lcs = [
                            cps_pool.tile([128, LC], F32, tag="cps", name=f"C{b}_{O}_{lc}")
                            for lc in range(NLC)
                        ]
                        for I in range(DT):
                            for lc in range(NLC):
                                nc.tensor.matmul(
                                    C_lcs[lc][:],
                                    WrT_sb[:, I, O, :],
                                    R_T[:, I, lc * LC : (lc + 1) * LC],
                                    start=(I == 0),
                                    stop=(I == DT - 1),
                                )
                        for lc in range(NLC):
                            nc.vector.tensor_scalar_add(
                                e_sbs[O][:, lc * LC : (lc + 1) * LC],
                                C_lcs[lc][:],
                                br_col[:, O : O + 1],
                            )
                            t_sb = ttpool.tile([128, LC], BF16, tag="ttsb", name="t_sb")
                            nc.scalar.activation(
                                t_sb[:],
                                C_lcs[lc][:],
                                mybir.ActivationFunctionType.Tanh,
                                bias=qbr_sb[:, O, b : b + 1],
                            )
                            t_grid[(O, lc)] = t_sb
                        # store this stripe immediately (partition p ->
                        # row 4p+O, affine stride); alternate rings
                        eng = nc.gpsimd if (b + O) % 2 == 0 else nc.scalar
                        eng.dma_start(
                            out=e_out[b, :, :].rearrange("(p r) l -> p r l", p=128)[
                                :, O, :
                            ],
                            in_=e_sbs[O][:],
                        )

                    for lc in range(NLC):
                        U_ps = ups_pool.tile([BLOC, LC], F32, tag="ups", name="U_ps")
                        for O in range(DT):
                            nc.tensor.matmul(
                                U_ps[:],
                                v_rep[:, O, :],
                                t_grid[(O, lc)][:],
                                start=(O == 0),
                                stop=(O == DT - 1),
                            )
                        nc.vector.tensor_copy(
                            logit_b[0:1, lc * LC : (lc + 1) * LC], U_ps[0:1, :]
                        )
                        if b == BLOC - 1:
                            nc.scalar.dma_start(
                                out=logits_out[b, lc * LC : (lc + 1) * LC],
                                in_=logit_b[0:1, lc * LC : (lc + 1) * LC],
                            )

                    if b < BLOC - 1:
                        nc.scalar.dma_start(out=logits_out[b, :], in_=logit_b[0:1, :])
                    del R_tiles[b]

    nc.compile()
    return nc


_CACHE: dict = {}


def _get_nc():
    if "nc" not in _CACHE:
        _CACHE["nc"] = build_nc()
    return _CACHE["nc"]


def kernel(**inputs) -> tuple[np.ndarray, np.ndarray]:
    query = np.ascontiguousarray(np.asarray(inputs["query"], dtype=np.float32))
    ref = np.asarray(inputs["ref"], dtype=np.float32)
    Wq = np.ascontiguousarray(np.asarray(inputs["Wq"], dtype=np.float32))
    bq = np.ascontiguousarray(np.asarray(inputs["bq"], dtype=np.float32))
    Wr = np.ascontiguousarray(np.asarray(inputs["Wr"], dtype=np.float32))
    br = np.ascontiguousarray(np.asarray(inputs["br"], dtype=np.float32))
    v = np.ascontiguousarray(np.asarray(inputs["v"], dtype=np.float32))

    in_maps = []
    for c in range(NCORES):
        sl = slice(c * BLOC, (c + 1) * BLOC)
        in_maps.append(
            {
                "query": np.ascontiguousarray(query[sl]),
                # shard + lay out as [B_loc, D, L] (the reference's ref_bdl
                # view) so the device reads D directly onto partitions
                "ref": np.ascontiguousarray(np.transpose(ref[:, sl, :], (1, 2, 0))),
                "Wq": Wq,
                "bq": bq,
                "Wr": Wr,
                "br": br,
                "v": v,
            }
        )

    res = run_bass_kernel_spmd(_get_nc(), in_maps, core_ids=list(range(NCORES)))
    _CACHE["last_result"] = res

    e = np.concatenate([res.results[c]["e"] for c in range(NCORES)], axis=0)
    logits = np.concatenate([res.results[c]["logits"] for c in range(NCORES)], axis=0)
    return (e, logits)


# revision 36
# speedup vs baseline: 1.1020x; 1.0299x over previous
"""Trainium2 Bass kernel for nn_Attention (additive-attention scoring).

Reference computation (per batch b):
    q[b]      = query[b] @ Wq.T + bq                       # [D]
    e[b,:,l]  = Wr @ ref[l,b,:] + br                       # [D, L]
    logits[b,l] = sum_o v[o] * tanh(q[b,o] + e[b,o,l])     # [L]
Returns (e, logits) with e: [B, D, L] f32, logits: [B, L] f32.

Strategy: data-parallel over batch - B=64 over 8 cores, weights replicated.
Each core's ref shard is laid out host-side as [B_loc, D, L] (the reference's
own ref_bdl view) during in_map construction, so the contraction dim D lands
on SBUF partitions with a plain strided DMA - no on-device transpose.
The conv matmul runs in float32r (full-rate fp32 on the PE at N>=256), f32
accumulate in PSUM. DVE adds br and stores e (f32); ACT computes
tanh(conv + q+bq+br) to bf16; a PE M=8 matmul against replicated-v reduces
over D for the logits.
"""

import os
import sys

import numpy as np

for _p in ("/root/.axon_site", "/root/.axon_site/_ro/trn_rl_repo",
           "/root/.axon_site/_ro/pypackages", "/opt/trn_rl_repo"):
    if os.path.isdir(_p) and _p not in sys.path:
        sys.path.append(_p)

import concourse.bass as bass  # noqa: E402,F401
import concourse.mybir as mybir  # noqa: E402
import concourse.tile as tile  # noqa: E402
from concourse import bacc  # noqa: E402
from concourse.bass_utils import run_bass_kernel_spmd  # noqa: E402
from concourse.masks import make_identity  # noqa: E402

F32 = mybir.dt.float32
F32R = mybir.dt.float32r
BF16 = mybir.dt.bfloat16

NCORES = 8
B = 64
BLOC = B // NCORES  # 8 batches per core
D = 512
L = 2048
DT = D // 128  # 4 partition tiles of the dim axis
LC = 512      # l-chunk processed per inner iteration
NLC = L // LC  # 4


def _preamble(nc, tc, singles, query, Wq, bq, Wr, br, v):
    """Load + transpose weights, compute q = Linear(query). Returns the
    SBUF residents used by the main loop."""
    identity = singles.tile([128, 128], F32)
    make_identity(nc, identity)

    # Per-partition columns of the small vectors: col[p, O] = vec[O*128+p]
    # stripe-permuted: col[p, r] = vec[4p + r] (matches the weight layout;
    # fully contiguous DMA)
    br_col = singles.tile([128, DT], F32)
    nc.gpsimd.dma_start(out=br_col[:], in_=br[:].rearrange("(p r) -> p r", p=128))
    bq_col = singles.tile([128, DT], F32)
    nc.gpsimd.dma_start(out=bq_col[:], in_=bq[:].rearrange("(p r) -> p r", p=128))
    v_col = singles.tile([128, DT], F32)
    nc.gpsimd.dma_start(out=v_col[:], in_=v[:].rearrange("(p r) -> p r", p=128))
    bqbr_col = singles.tile([128, DT], F32)
    nc.vector.tensor_add(bqbr_col[:], bq_col[:], br_col[:])

    # v replicated across BLOC columns -> stationary operand of the
    # v-reduce matmul (M=BLOC so output rows land on partitions 0..7).
    zero_bloc = singles.tile([128, BLOC], F32)
    nc.vector.memset(zero_bloc[:], 0.0)
    v_rep = singles.tile([128, DT, BLOC], BF16)
    for O in range(DT):
        nc.vector.tensor_scalar_add(v_rep[:, O, :], zero_bloc[:], v_col[:, O : O + 1])

    # Transposed weights, stripe-permuted output rows:
    # WT[i_part, I, r, p] = W[4p + r, I*128 + i_part] - a matmul with this
    # lhsT produces PSUM partition p holding output row o = 4p + r; the
    # permutation is undone by an affine store AP (stride 4 rows).
    WrT_sb = singles.tile([128, DT, DT, 128], F32R)
    qbr_sb = singles.tile([128, DT, BLOC], F32)
    # q-path inputs: loads issued now, PE work deferred into batch 0 so the
    # first conv matmuls aren't queued behind the q chain
    q_nat = singles.tile([BLOC, D], F32)
    nc.scalar.dma_start(out=q_nat[:], in_=query[:, :])
    Wq_nat = singles.tile([128, DT, D], F32)
    WqT_sb = singles.tile([128, DT, DT, 128], F32)
    qT_sb = singles.tile([128, DT, BLOC], F32)

    with (
        tc.tile_pool(name="pre", bufs=1) as pre,
        tc.tile_pool(name="pre_ps", bufs=2, space="PSUM") as pre_ps,
    ):
        W_nat = pre.tile([128, DT, D], F32, tag="wnat", bufs=1, name="W_nat")
        # o-rows 4p..4p+3 are DRAM-contiguous: one 8 KB run/partition
        nc.scalar.dma_start(
            out=W_nat[:], in_=Wr[:, :].rearrange("(p r) i -> p r i", p=128)
        )
        nc.scalar.dma_start(
            out=Wq_nat[:], in_=Wq[:, :].rearrange("(p r) i -> p r i", p=128)
        )
        for r in range(DT):
            W_ps = pre_ps.tile([128, D], F32, tag="wps", name="W_ps")
            for I in range(DT):
                nc.tensor.transpose(
                    W_ps[:, I * 128 : (I + 1) * 128],
                    W_nat[:, r, I * 128 : (I + 1) * 128],
                    identity[:],
                )
            nc.vector.tensor_copy(
                WrT_sb[:, :, r, :],
                W_ps[:].rearrange("p (I c) -> p I c", I=DT),
            )

    def emit_q(cps_pool):
        for r in range(DT):
            W_ps = cps_pool.tile([128, D], F32, tag="cps", name=f"Wq_ps{r}")
            for I in range(DT):
                nc.tensor.transpose(
                    W_ps[:, I * 128 : (I + 1) * 128],
                    Wq_nat[:, r, I * 128 : (I + 1) * 128],
                    identity[:],
                )
            nc.vector.tensor_copy(
                WqT_sb[:, :, r, :],
                W_ps[:].rearrange("p (I c) -> p I c", I=DT),
            )
        qT_ps = cps_pool.tile([128, DT, BLOC], F32, tag="cps", name="qT_ps")
        for I in range(DT):
            nc.tensor.transpose(
                qT_ps[:, I, :],
                q_nat[:, I * 128 : (I + 1) * 128],
                identity[0:BLOC, 0:BLOC],
            )
        nc.vector.tensor_copy(qT_sb[:], qT_ps[:])
        for r in range(DT):
            q_ps = cps_pool.tile([128, BLOC], F32, tag="cps", name=f"q_ps{r}")
            for I in range(DT):
                nc.tensor.matmul(
                    q_ps[:],
                    WqT_sb[:, I, r, :],
                    qT_sb[:, I, :],
                    start=(I == 0),
                    stop=(I == DT - 1),
                )
            # qbr = q + bq + br (the tanh bias; e's br is added separately)
            nc.vector.tensor_scalar_add(qbr_sb[:, r, :], q_ps[:], bqbr_col[:, r : r + 1])

    return identity, br_col, v_rep, WrT_sb, qbr_sb, emit_q


def build_nc():
    nc = bacc.Bacc(None)

    query = nc.declare_dram_parameter("query", [BLOC, D], F32, isOutput=False)
    # ref arrives pre-permuted host-side to [B_loc, D, L] (the reference's own
    # ref_bdl view) so D lands on partitions with a plain strided load
    ref = nc.declare_dram_parameter("ref", [BLOC, D, L], F32R, isOutput=False)
    Wq = nc.declare_dram_parameter("Wq", [D, D], F32, isOutput=False)
    bq = nc.declare_dram_parameter("bq", [D], F32, isOutput=False)
    Wr = nc.declare_dram_parameter("Wr", [D, D], F32, isOutput=False)
    br = nc.declare_dram_parameter("br", [D], F32, isOutput=False)
    v = nc.declare_dram_parameter("v", [D], F32, isOutput=False)
    e_out = nc.declare_dram_parameter("e", [BLOC, D, L], F32, isOutput=True)
    logits_out = nc.declare_dram_parameter("logits", [BLOC, L], F32, isOutput=True)

    with tile.TileContext(nc) as tc:
        with (
            tc.tile_pool(name="singles", bufs=1) as singles,
            tc.tile_pool(name="rpool", bufs=3) as rpool,
            tc.tile_pool(name="epool", bufs=6) as epool,
            tc.tile_pool(name="ttpool", bufs=12) as ttpool,
            tc.tile_pool(name="lpool", bufs=2) as lpool,
        ):
            def issue_load(b):
                # four fully-contiguous 1 MB stripe loads: i = I*128 + p
                R_T = rpool.tile([128, DT, L], F32R, tag="rT", name=f"RT{b}")
                for I in range(DT):
                    nc.sync.dma_start(
                        out=R_T[:, I, :], in_=ref[b, I * 128 : (I + 1) * 128, :]
                    )
                return R_T

            # start streaming ref before the preamble so the first conv
            # matmuls are not gated on the preamble pool-exit barrier
            R_tiles = {0: issue_load(0), 1: issue_load(1)}

            identity, br_col, v_rep, WrT_sb, qbr_sb, emit_q = _preamble(
                nc, tc, singles, query, Wq, bq, Wr, br, v
            )

            with (
                tc.tile_pool(name="cps", bufs=6, space="PSUM") as cps_pool,
                tc.tile_pool(name="ups", bufs=2, space="PSUM") as ups_pool,
            ):
                for b in range(BLOC):
                    if b + 2 < BLOC:
                        R_tiles[b + 2] = issue_load(b + 2)
                    R_T = R_tiles[b]
                    e_sbs = [
                        epool.tile([128, L], F32, tag="esb", name=f"esb{b}_{O}")
                        for O in range(DT)
                    ]
                    # All BLOC rows of U_ps are identical (v replicated), so
                    # logits live on partition 0 - PSUM reads must start at an
                    # aligned partition, so row b is never read directly.
                    logit_b = lpool.tile([1, L], F32, tag="lsb", name=f"lsb{b}")

                    # conv: keep each weight block stationary across all 4
                    # l-chunks (one LDWEIGHTS per 4 matmuls - FWL is off for
                    # f32r, so weight loads are the PE overhead to amortize)
                    t_grid = {}
                    for O in range(DT):
                        C_lcs = [
                            cps_pool.tile([128, LC], F32, tag="cps", name=f"C{b}_{O}_{lc}")
                            for lc in range(NLC)
                        ]
                        for I in range(DT):
                            for lc in range(NLC):
                                nc.tensor.matmul(
                                    C_lcs[lc][:],
                                    WrT_sb[:, I, O, :],
                                    R_T[:, I, lc * LC : (lc + 1) * LC],
                                    start=(I == 0),
                                    stop=(I == DT - 1),
                                )
                        if b == 0 and O == 0:
                            # q-path PE work rides behind the first conv group
                            emit_q(cps_pool)
                        for lc in range(NLC):
                            nc.vector.tensor_scalar_add(
                                e_sbs[O][:, lc * LC : (lc + 1) * LC],
                                C_lcs[lc][:],
                                br_col[:, O : O + 1],
                            )
                            t_sb = ttpool.tile([128, LC], BF16, tag="ttsb", name="t_sb")
                            nc.scalar.activation(
                                t_sb[:],
                                C_lcs[lc][:],
                                mybir.ActivationFunctionType.Tanh,
                                bias=qbr_sb[:, O, b : b + 1],
                            )
                            t_grid[(O, lc)] = t_sb
                        # store this stripe immediately (partition p ->
                        # row 4p+O, affine stride); alternate rings
                        eng = nc.gpsimd if (b + O) % 2 == 0 else nc.scalar
                        eng.dma_start(
                            out=e_out[b, :, :].rearrange("(p r) l -> p r l", p=128)[
                                :, O, :
                            ],
                            in_=e_sbs[O][:],
                        )

                    for lc in range(NLC):
                        U_ps = ups_pool.tile([BLOC, LC], F32, tag="ups", name="U_ps")
                        for O in range(DT):
                            nc.tensor.matmul(
                                U_ps[:],
                                v_rep[:, O, :],
                                t_grid[(O, lc)][:],
                                start=(O == 0),
                                stop=(O == DT - 1),
                            )
                        nc.vector.tensor_copy(
                            logit_b[0:1, lc * LC : (lc + 1) * LC], U_ps[0:1, :]
                        )
                        if b == BLOC - 1:
                            nc.scalar.dma_start(
                                out=logits_out[b, lc * LC : (lc + 1) * LC],
                                in_=logit_b[0:1, lc * LC : (lc + 1) * LC],
                            )

                    if b < BLOC - 1:
                        nc.scalar.dma_start(out=logits_out[b, :], in_=logit_b[0:1, :])
                    del R_tiles[b]

    nc.compile()
    return nc


_CACHE: dict = {}


def _get_nc():
    if "nc" not in _CACHE:
        _CACHE["nc"] = build_nc()
    return _CACHE["nc"]


def kernel(**inputs) -> tuple[np.ndarray, np.ndarray]:
    query = np.ascontiguousarray(np.asarray(inputs["query"], dtype=np.float32))
    ref = np.asarray(inputs["ref"], dtype=np.float32)
    Wq = np.ascontiguousarray(np.asarray(inputs["Wq"], dtype=np.float32))
    bq = np.ascontiguousarray(np.asarray(inputs["bq"], dtype=np.float32))
    Wr = np.ascontiguousarray(np.asarray(inputs["Wr"], dtype=np.float32))
    br = np.ascontiguousarray(np.asarray(inputs["br"], dtype=np.float32))
    v = np.ascontiguousarray(np.asarray(inputs["v"], dtype=np.float32))

    in_maps = []
    for c in range(NCORES):
        sl = slice(c * BLOC, (c + 1) * BLOC)
        in_maps.append(
            {
                "query": np.ascontiguousarray(query[sl]),
                # shard + lay out as [B_loc, D, L] (the reference's ref_bdl
                # view) so the device reads D directly onto partitions
                "ref": np.ascontiguousarray(np.transpose(ref[:, sl, :], (1, 2, 0))),
                "Wq": Wq,
                "bq": bq,
                "Wr": Wr,
                "br": br,
                "v": v,
            }
        )

    res = run_bass_kernel_spmd(_get_nc(), in_maps, core_ids=list(range(NCORES)))
    _CACHE["last_result"] = res

    e = np.concatenate([res.results[c]["e"] for c in range(NCORES)], axis=0)
    logits = np.concatenate([res.results[c]["logits"] for c in range(NCORES)], axis=0)
    return (e, logits)
